# revision 8
# baseline (speedup 1.0000x reference)
"""nn_Cvx_ShortestPathNet — TRN2 Bass kernel, 8-core pure data parallelism.

Math (collapsed Dykstra, c folded into G via a constant-1 row):
    G = A' pinv(AA') A, c = b' pinv(AA') A.  Augment Gp[760,:] = -c,
    Gp[760,760] = 1, and pin t[760] = 1 (negw[760] = 1), so
        t @ Gp == t @ G - c.
    Iterate t <- max(t @ Gp, -w) 99 times from t = -w; output
    y = max(-w - t@Gp, 0).

Per core (batch 32 columns, edge dim 760 -> 768 = 6x128 tiles):
the PE is weight-load bound (~26.6ns per 128x128 fp16 LDWEIGHTS, 36
blocks/iter => ~958ns/iter floor).  The 36 matmuls per iteration are
slot-scheduled so the 6 psum chains complete staggered; each chain's
max() runs on DVE as soon as that chain finishes, and next-iteration
matmuls that contract tile k are placed late enough that t[k] is always
ready — PE never stalls on the elementwise path.
"""

import json
import numpy as np

import concourse.bass as bass
import concourse.mybir as mybir
import concourse.tile as tile
from concourse.bass_utils import run_bass_kernel_spmd

F32 = mybir.dt.float32
F16 = mybir.dt.float16
AT = mybir.AluOpType
AF = mybir.ActivationFunctionType

JT = 6          # 768/128 edge-dim tiles
BL = 32         # batch rows per core
HT = 5          # 640/128 hidden tiles
K_ITERS = 100
N_CORES = 8
N2 = 760

# 36 (j, k) matmul slots per iteration — optimal cyclic schedule.
# Chain j's last matmul sits at slot e_j in {18,22,25,29,32,36}; every
# consumer of t[k] (produced by chain k's max() right after e_k) is
# placed >= e_k - 18 slots, the best possible spread (slack 5d = 36-5d
# => d = 3.6).  With the cross-engine latency L ~= 500ns and 26.6ns per
# slot, t[k] is ready before its first next-iteration consumer and the
# PE never stalls.
SLOTS = [(0, 0), (1, 0), (2, 0), (0, 1), (1, 1), (2, 1), (0, 2), (1, 2),
         (0, 3), (2, 2), (1, 3), (0, 4), (2, 3), (1, 4), (3, 0), (0, 5),
         (2, 4), (3, 1), (3, 2), (1, 5), (3, 3), (3, 4), (4, 0), (2, 5),
         (4, 1), (4, 2), (4, 3), (3, 5), (5, 0), (4, 4), (5, 1), (4, 5),
         (5, 2), (5, 3), (5, 4), (5, 5)]

# ---------------------------------------------------------------------------
# This container's walrus build rejects instructions carrying more than one
# sync-wait. Split any multi-wait instruction at the BIR-JSON level: insert
# same-engine NoOps before it, each carrying one of the extra waits (waits
# are sem-ge, so order is irrelevant).
_orig_to_json_bytes = bass.Bass.to_json_bytes
_ctr = [0]


def _split_waits_json(raw: bytes) -> bytes:
    j = json.loads(raw)
    changed = False
    for fn in j.get("functions", []):
        # Running count of each engine-counter semaphore (by sem id),
        # walking blocks/instructions in listed (= emission) order.  A wait
        # by engine E on E's own counter with threshold <= count-4 is
        # trivially satisfied (engines complete in order) — drop it instead
        # of spending a NoOp on it.
        own_count: dict[int, float] = {}
        eng_sem: dict[str, int] = {}
        for bb in fn.get("blocks", []):
            for inst in bb.get("instructions", []):
                si = inst.get("sync_info") or {}
                for u in si.get("on_update") or []:
                    if u.get("update_mode") == "sem-inc" and str(
                            u.get("ant_name", "")).startswith(inst["engine"] + "_"):
                        eng_sem[inst["engine"]] = u["id"]
        for bb in fn.get("blocks", []):
            out = []
            for inst in bb.get("instructions", []):
                si = inst.get("sync_info") or {}
                waits = si.get("on_wait") or []
                if waits:
                    own = eng_sem.get(inst["engine"])
                    kept = []
                    for w in waits:
                        if (w.get("id") == own
                                and w.get("wait_mode") == "sem-ge-imm"
                                and w.get("wait_value", 1 << 60)
                                <= own_count.get(own, 0) - 4):
                            changed = True
                            continue
                        kept.append(w)
                    waits = kept
                    si["on_wait"] = waits
                if len(waits) > 1:
                    changed = True
                    for w in waits[:-1]:
                        _ctr[0] += 1
                        out.append({
                            "debug": inst.get("debug", 0),
                            "engine": inst["engine"],
                            "ins": [], "outs": [],
                            "name": f"I-waitsplit-{_ctr[0]}",
                            "opcode": "NoOp",
                            "sync_info": {"on_wait": [w], "on_update": []},
                        })
                    si["on_wait"] = waits[-1:]
                out.append(inst)
                for u in si.get("on_update") or []:
                    if u.get("update_mode") == "sem-inc":
                        own_count[u["id"]] = own_count.get(u["id"], 0) + u.get(
                            "update_value", 1)
            bb["instructions"] = out
    return json.dumps(j).encode() if changed else raw


def _patched_to_json_bytes(self, *a, **k):
    return _split_waits_json(_orig_to_json_bytes(self, *a, **k))


bass.Bass.to_json_bytes = _patched_to_json_bytes


def _build(k_iters=K_ITERS):
    nc = bass.Bass("TRN2", target_bir_lowering=False, debug=False,
                   num_devices=N_CORES)

    g_mat = nc.dram_tensor("g_mat", [128, JT * JT * 128], F16, kind="ExternalInput").ap()
    w2t = nc.dram_tensor("w2t", [128, HT * JT * 128], F16, kind="ExternalInput").ap()
    w1 = nc.dram_tensor("w1", [64, HT * 128], F16, kind="ExternalInput").ap()
    dt_in = nc.dram_tensor("dt_in", [64, BL], F16, kind="ExternalInput").ap()
    b1c = nc.dram_tensor("b1c", [128, HT], F32, kind="ExternalInput").ap()
    nb2c = nc.dram_tensor("nb2c", [128, JT], F32, kind="ExternalInput").ap()
    y_out = nc.dram_tensor("y_out", [128, JT * BL], F32, kind="ExternalOutput").ap()

    with tile.TileContext(nc) as tc:
        with (
            tc.tile_pool(name="const", bufs=1) as cpool,
            tc.tile_pool(name="state", bufs=2) as spool,
            tc.tile_pool(name="psum", bufs=2, space="PSUM") as ppool,
            tc.tile_pool(name="psum1", bufs=1, space="PSUM") as ppool1,
        ):
            dT_sb = cpool.tile([64, BL], F16)
            nc.sync.dma_start(out=dT_sb[:], in_=dt_in[:])
            w1_sb = cpool.tile([64, HT * 128], F16)
            nc.sync.dma_start(out=w1_sb[:], in_=w1[:])
            b1c_sb = cpool.tile([128, HT], F32)
            nc.sync.dma_start(out=b1c_sb[:], in_=b1c[:])
            nb2c_sb = cpool.tile([128, JT], F32)
            nc.sync.dma_start(out=nb2c_sb[:], in_=nb2c[:])
            w2_sb = cpool.tile([128, HT * JT * 128], F16)
            nc.sync.dma_start(out=w2_sb[:], in_=w2t[:])
            # G on the SWDGE path so it overlaps the W2 load
            G_sb = cpool.tile([128, JT * JT * 128], F16)
            nc.gpsimd.dma_start(out=G_sb[:], in_=g_mat[:])

            # MLP: h = leaky_relu(d@W1 + b1), negw = -(h@W2 + b2)
            h_sb = cpool.tile([128, HT * BL], F16)
            for m in range(HT):
                ph = ppool.tile([128, BL], F32, tag="mlp")
                nc.tensor.matmul(out=ph[:], lhsT=w1_sb[:, m * 128:(m + 1) * 128],
                                 rhs=dT_sb[:], start=True, stop=True)
                pre = spool.tile([128, BL], F32, tag="pre", name=f"pre{m}")
                nc.scalar.activation(out=pre[:], in_=ph[:], func=AF.Identity,
                                     bias=b1c_sb[:, m:m + 1], scale=1.0)
                # leaky relu: max(x, 0.1x) on DVE (Lrelu alpha is hardcoded
                # to 0.01 in this compiler build)
                nc.vector.scalar_tensor_tensor(
                    out=h_sb[:, m * BL:(m + 1) * BL], in0=pre[:],
                    scalar=0.1, in1=pre[:], op0=AT.mult, op1=AT.max)
            negw = cpool.tile([128, JT * BL], F16)
            for j in range(JT):
                pw = ppool.tile([128, BL], F32, tag="mlp")
                for k2 in range(HT):
                    nc.tensor.matmul(
                        out=pw[:],
                        lhsT=w2_sb[:, (k2 * JT + j) * 128:(k2 * JT + j + 1) * 128],
                        rhs=h_sb[:, k2 * BL:(k2 + 1) * BL],
                        start=(k2 == 0), stop=(k2 == HT - 1))
                nc.scalar.activation(out=negw[:, j * BL:(j + 1) * BL], in_=pw[:],
                                     func=AF.Identity, bias=nb2c_sb[:, j:j + 1],
                                     scale=-1.0)

            z_sb = cpool.tile([128, JT * BL], F32)
            y_sb = cpool.tile([128, JT * BL], F32)

            t_read = negw  # iteration 0 reads -w directly
            for it in range(k_iters):
                # one PSUM bank per chain: only one open accumulation group
                # is allowed per bank, and the 6 chains interleave
                pss = [ppool1.tile([128, BL], F32, tag=f"ps{j}",
                                  name=f"ps{it}_{j}") for j in range(JT)]
                last = (it == k_iters - 1)
                if not last:
                    t_nxt = spool.tile([128, JT * BL], F16, tag="t",
                                       name=f"t{it}")
                seen = [0] * JT
                for s, (j, k) in enumerate(SLOTS, start=1):
                    kp = 121 if k == 5 else 128
                    # pin the schedule: virtual timestamps force the Tile
                    # scheduler to keep exactly this interleave
                    tc.tile_set_cur_wait(it * 40 + s)
                    nc.tensor.matmul(
                        out=pss[j][:],
                        lhsT=G_sb[0:kp, (k * JT + j) * 128:(k * JT + j + 1) * 128],
                        rhs=t_read[0:kp, k * BL:(k + 1) * BL],
                        start=(seen[j] == 0), stop=(seen[j] == JT - 1))
                    seen[j] += 1
                    if seen[j] == JT:
                        sl = slice(j * BL, (j + 1) * BL)
                        tc.tile_set_cur_wait(it * 40 + s + 0.5)
                        if not last:
                            nc.vector.tensor_tensor(
                                out=t_nxt[:, sl], in0=pss[j][:],
                                in1=negw[:, sl], op=AT.max)
                        else:
                            nc.vector.tensor_tensor(
                                out=z_sb[:, sl], in0=pss[j][:],
                                in1=negw[:, sl], op=AT.subtract)
                if not last:
                    t_read = t_nxt
            tc.cur_wait_ts = None

            # z = psum - negw, so y = relu(-z)
            for cpair in range(3):
                sl = slice(cpair * 2 * BL, (cpair + 1) * 2 * BL)
                nc.scalar.activation(out=y_sb[:, sl], in_=z_sb[:, sl],
                                     func=AF.Relu, scale=-1.0)
            nc.sync.dma_start(out=y_out[:], in_=y_sb[:])
    return nc


def _host_prepare(d, W1, b1, W2, b2, A, b_eq):
    A64 = A.astype(np.float64)
    M = np.linalg.pinv(A64 @ A64.T)
    G = A64.T @ M @ A64
    c = (b_eq.astype(np.float64) @ M) @ A64

    n2 = A.shape[1]
    NP = JT * 128
    G_pad = np.zeros((NP, NP), np.float64)
    G_pad[:n2, :n2] = G
    G_pad[n2, :n2] = -c          # constant-1 row folds the -c shift in
    G_pad[n2, n2] = 1.0

    g_sb = (G_pad.reshape(JT, 128, JT, 128).transpose(1, 0, 2, 3)
            .reshape(128, JT * JT * 128)).astype(np.float16)

    HID = W1.shape[1]
    W2_pad = np.zeros((HID, NP), np.float64)
    W2_pad[:, :n2] = W2.astype(np.float64)
    w2_sb = (W2_pad.reshape(HT, 128, JT, 128).transpose(1, 0, 2, 3)
             .reshape(128, HT * JT * 128)).astype(np.float16)
    b1c = b1.reshape(HT, 128).T.astype(np.float32).copy()
    b2_pad = np.zeros(NP, np.float32)
    b2_pad[:n2] = b2
    b2_pad[n2] = -1.0            # => negw[760] = 1 (the constant-1 lane)
    nb2c = (-b2_pad).reshape(JT, 128).T.astype(np.float32).copy()

    shared = {"g_mat": g_sb, "w2t": w2_sb, "w1": W1.astype(np.float16),
              "b1c": b1c, "nb2c": nb2c}
    B = d.shape[0]
    bl = B // N_CORES
    in_maps = []
    for i in range(N_CORES):
        dT = d[i * bl:(i + 1) * bl, :].T.astype(np.float16).copy()
        in_maps.append({**shared, "dt_in": dT})
    return in_maps


_nc_cache = {}


def kernel(d, W1, b1, W2, b2, A, b_eq):
    d = np.asarray(d, np.float32)
    W1 = np.asarray(W1, np.float32)
    b1 = np.asarray(b1, np.float32)
    W2 = np.asarray(W2, np.float32)
    b2 = np.asarray(b2, np.float32)
    A = np.asarray(A, np.float32)
    b_eq = np.asarray(b_eq, np.float32)

    if "nc" not in _nc_cache:
        _nc_cache["nc"] = _build()
    nc = _nc_cache["nc"]

    in_maps = _host_prepare(d, W1, b1, W2, b2, A, b_eq)
    res = run_bass_kernel_spmd(nc, in_maps, list(range(N_CORES)))

    outs = []
    for r in res.results:
        y = (r["y_out"].reshape(128, JT, BL).transpose(2, 1, 0)
             .reshape(BL, JT * 128))
        outs.append(y[:, :N2])
    return np.concatenate(outs, axis=0).astype(np.float32)
